# revision 21
# baseline (speedup 1.0000x reference)
"""Trainium2 Bass kernel for nn_MHA_43095701848407.

MHA forward: qkv = x @ W_qkv, RoPE on q/k, causal softmax attention,
y @ W_proj.  B=4, T=2048, C=2048, 16 heads, head_dim=128, fp32.

Sharding (8 cores): tensor-parallel over heads (4 shards x 4 heads) x
data-parallel over batch (2 groups x 2 batches).  core = group*4 + shard.

v3: fused single pipeline per batch, all intermediates SBUF-resident,
bf16 matmul operands (fp32 psum).  RoPE via a permutation matmul
(rot^T = PT.T @ qk^T), rot pairs packed into one psum pair-tile.
Attention processes key-blocks in PAIRS: two score matmuls into one
2-bank psum tile, a single 1024-wide exp on ACT, then l,l and y,y
emitted back-to-back into their accumulation banks -- minimizing PE
psum-bank switches (each switch costs the ~173ns SBUF access-latency
overlap) and halving ACT per-instruction overhead.  Causal diagonal
blocks keep per-half exp + a gpsimd affine_select on the 128-wide
triangle.  softmax 1/l is broadcast via a DRAM bounce; y is evacuated
normalized; the output projection (co-pairs) is zippered into the
following tile's (or next batch's) instruction stream.
Host sums the 4 head-shard partials per batch and transposes back.

Self-contained: shapes/sharding hardcoded; inputs full-size numpy arrays.
"""

import math
import os
import sys
import types

import ml_dtypes
import numpy as np

import concourse.bass as bass
import concourse.mybir as mybir
import concourse.tile as tile
from concourse import bacc
from concourse.bass_utils import run_bass_kernel_spmd

F32 = mybir.dt.float32
F32R = mybir.dt.float32r
BF16 = mybir.dt.bfloat16
AF = mybir.ActivationFunctionType
ALU = mybir.AluOpType
BF16NP = ml_dtypes.bfloat16

# Problem shape (hardcoded per contract)
B, T, C = 4, 2048, 2048
H, HD = 16, 128
NCORES = 8
BGROUPS, HSHARDS = 2, 4  # batch groups x head shards
B_LOC = B // BGROUPS  # 2 batches per core
H_LOC = H // HSHARDS  # 4 heads per core
FQK = H_LOC * HD  # 512 features for q (and for k)
FV = H_LOC * HD  # 512 features for v
F_ALL = 3 * H_LOC * HD  # 1536 qkv features per core
KO = C // 128  # 16 contraction chunks
TSLAB = 512
NSLAB = T // TSLAB  # 4 t-slabs per batch
QTILE = 512
NQT = T // QTILE  # 4 q-tiles
NKB = T // 128  # 16 key blocks
SCALE = 1.0 / math.sqrt(HD)
HH = HD // 2  # 64: half head dim (even/odd split)

_CACHED = {}


def _install_ntff_hook():
    """Register the axon NTFF profile hook (container's antenv lacks it)."""
    if "antenv.axon_hooks" in sys.modules:
        return
    try:
        mod = types.ModuleType("antenv.axon_hooks")
        holder = [None]
        mod.set_axon_ntff_profile_hook = lambda h: holder.__setitem__(0, h)
        mod.get_axon_ntff_profile_hook = lambda: holder[0]
        sys.modules["antenv.axon_hooks"] = mod
        import antenv

        antenv.axon_hooks = mod
        if "/root/.axon_site" not in sys.path:
            sys.path.insert(0, "/root/.axon_site")
        from trn_agent_boot.trn_boot import _ntff_profile_via_ctypes

        mod.set_axon_ntff_profile_hook(
            _ntff_profile_via_ctypes("/opt/axon/libaxon_pjrt.so")
        )
    except Exception:
        sys.modules.pop("antenv.axon_hooks", None)


def build_nc():
    nc = bacc.Bacc("TRN2", target_bir_lowering=False, debug=False)

    x_t = nc.dram_tensor("x_t", [B_LOC, C, T], BF16, kind="ExternalInput").ap()
    w_qkv = nc.dram_tensor("w_qkv", [C, F_ALL], BF16, kind="ExternalInput").ap()
    w_proj = nc.dram_tensor(
        "w_proj", [HD, H_LOC, C], BF16, kind="ExternalInput"
    ).ap()
    sin_t = nc.dram_tensor("sin_t", [HD, T], F32, kind="ExternalInput").ap()
    cos_t = nc.dram_tensor("cos_t", [HD, T], F32, kind="ExternalInput").ap()
    pt = nc.dram_tensor("pt", [HD, HD], F32R, kind="ExternalInput").ap()
    ones_col = nc.dram_tensor("ones_col", [128, 1], BF16, kind="ExternalInput").ap()
    out_t = nc.dram_tensor("out_t", [B_LOC, C, T], F32, kind="ExternalOutput").ap()

    with tile.TileContext(nc) as tc:
        with nc.allow_low_precision(reason="bf16 matmul operands by design"):
            _emit(nc, tc, x_t, w_qkv, w_proj, sin_t, cos_t, pt, ones_col, out_t)
    nc.compile()
    return nc


def _emit(nc, tc, x_t, w_qkv, w_proj, sin_t, cos_t, pt, ones_col, out_t):
    with (
        tc.tile_pool(name="consts", bufs=1) as consts,
        tc.tile_pool(name="xpool", bufs=2) as xpool,
        tc.tile_pool(name="rawpool", bufs=3) as rawpool,
        tc.tile_pool(name="t1pool", bufs=2) as t1pool,
        tc.tile_pool(name="t2pool", bufs=2) as t2pool,
        tc.tile_pool(name="ppool", bufs=5) as ppool,
        tc.tile_pool(name="ypool", bufs=2) as ypool,
        tc.tile_pool(name="lpool", bufs=2) as lpool,
        tc.tile_pool(name="bcpool", bufs=2) as bcpool,
        tc.tile_pool(name="opool", bufs=3) as opool,
        tc.tile_pool(name="nbounce", bufs=4, space="DRAM") as nbounce,
    ):
        # ---- resident tiles ----
        w_sb = consts.tile([128, KO, F_ALL], BF16)
        wp_sb = consts.tile([128, H_LOC, C], BF16)
        sin_sb = consts.tile([HD, T], F32)
        cos_sb = consts.tile([HD, T], F32)
        pt_sb = consts.tile([HD, HD], F32R)
        ones_sb = consts.tile([128, 1], BF16)
        kt_sb = consts.tile([128, H_LOC, T], BF16)  # k^T per head
        qt_sb = consts.tile([128, H_LOC, T], BF16)  # q^T per head
        v_sb = consts.tile([128, NKB, FV], BF16)  # v natural [t, f]

        w_src = w_qkv.rearrange("(ko p) f -> p ko f", p=128)
        x3 = [x_t[b].rearrange("(ko p) t -> p ko t", p=128) for b in range(B_LOC)]

        # ---- initial loads: pace W/x chunk arrival with first-slab use ----
        nc.scalar.dma_start(ones_sb, ones_col)
        nc.scalar.dma_start(pt_sb, pt)
        nc.scalar.dma_start(sin_sb[:, 0:TSLAB], sin_t[:, 0:TSLAB])
        nc.scalar.dma_start(cos_sb[:, 0:TSLAB], cos_t[:, 0:TSLAB])
        x_first = xpool.tile([128, KO, TSLAB], BF16, name="x_sb")
        for ko in range(KO):
            nc.sync.dma_start(w_sb[:, ko, 0:384], w_src[:, ko, 0:384])
            nc.scalar.dma_start(x_first[:, ko, :], x3[0][:, ko, 0:TSLAB])
        for c0, c1 in ((384, 768), (768, 1024), (1024, 1536)):
            for i, ko in enumerate(range(KO)):
                eng = nc.sync if i % 2 == 0 else nc.scalar
                eng.dma_start(w_sb[:, ko, c0:c1], w_src[:, ko, c0:c1])
        x_second = xpool.tile([128, KO, TSLAB], BF16, name="x_sb")
        nc.scalar.dma_start(x_second, x3[0][:, :, TSLAB : 2 * TSLAB])
        nc.scalar.dma_start(sin_sb[:, TSLAB:], sin_t[:, TSLAB:])
        nc.scalar.dma_start(cos_sb[:, TSLAB:], cos_t[:, TSLAB:])
        nc.scalar.dma_start(wp_sb, w_proj)

        # ---------- helpers ----------
        def rope_raw(ps_half):
            """Evacuate a qk psum chunk for the rotate matmul + combines."""
            raw = rawpool.tile([128, TSLAB], F32R, name="raw")
            nc.vector.tensor_copy(raw, ps_half)
            return raw

        def rope_finish(rotps, item):
            """Deferred: rot^T = PT.T @ raw on PE, then
            roped = raw*cos + rot*sin -> bf16 into qt/kt."""
            raw, f, tsl = item
            rot_ps = rotps.tile([128, TSLAB], F32, name="rot_ps")
            nc.tensor.matmul(rot_ps, pt_sb, raw, start=True, stop=True)
            t1 = t1pool.tile([128, TSLAB], F32, name="t1")
            nc.gpsimd.tensor_tensor(t1, raw, cos_sb[:, tsl], ALU.mult)
            t2 = t2pool.tile([128, TSLAB], F32, name="t2")
            nc.vector.tensor_tensor(t2, rot_ps, sin_sb[:, tsl], ALU.mult)
            dest = qt_sb[:, f, tsl] if f < H_LOC else kt_sb[:, f - H_LOC, tsl]
            nc.vector.tensor_tensor(dest, t1, t2, ALU.add)

        def emit_slab(b, js, x_sb, proj_queue, qkps, rotps, vpsp, first=False):
            tsl = slice(js * TSLAB, (js + 1) * TSLAB)
            pending = []  # rope items deferred one qk group

            def alloc2_o():
                return [
                    qkps.tile([128, TSLAB], F32, name="qk_ps"),
                    qkps.tile([128, TSLAB], F32, name="qk_ps"),
                ]

            if first:
                # ko-outer over f-triples so compute starts when the
                # first W/x ko-chunks land
                for fg in ([0, 1, 2], [3, 4, 5], [6, 7]):
                    pss = {
                        f: qkps.tile([128, TSLAB], F32, name="qk_ps") for f in fg
                    }
                    for ko in range(KO):
                        for f in fg:
                            nc.tensor.matmul(
                                pss[f],
                                w_sb[:, ko, f * 128 : (f + 1) * 128],
                                x_sb[:, ko, :],
                                start=(ko == 0),
                                stop=(ko == KO - 1),
                            )
                    for f in fg:
                        if pending:
                            rope_finish(rotps, pending.pop(0))
                        pending.append((rope_raw(pss[f]), f, tsl))
            else:
                for f in range(2 * H_LOC):
                    ps = qkps.tile([128, TSLAB], F32, name="qk_ps")
                    for ko in range(KO):
                        nc.tensor.matmul(
                            ps,
                            w_sb[:, ko, f * 128 : (f + 1) * 128],
                            x_sb[:, ko, :],
                            start=(ko == 0),
                            stop=(ko == KO - 1),
                        )
                    pending.append((rope_raw(ps), f, tsl))
                    if len(pending) > 1:
                        rope_finish(rotps, pending.pop(0))
                    if f % 2 == 1 and proj_queue:
                        proj_queue.pop(0)(alloc2_o)
            # v natural [t, f] chunks
            for tb in range(TSLAB // 128):
                vps = vpsp.tile([128, FV], F32, name="v_ps")
                for ko in range(KO):
                    nc.tensor.matmul(
                        vps,
                        x_sb[:, ko, tb * 128 : (tb + 1) * 128],
                        w_sb[:, ko, 2 * FQK :],
                        start=(ko == 0),
                        stop=(ko == KO - 1),
                    )
                if pending:
                    rope_finish(rotps, pending.pop(0))
                if proj_queue:
                    proj_queue.pop(0)(alloc2_o)
                nc.vector.tensor_copy(v_sb[:, js * 4 + tb, :], vps)
            while pending:
                rope_finish(rotps, pending.pop(0))

        def emit_attn_tile(b, jt, proj_queue, s2ps, yps, lps):
            """Attention q-tile jt, all local heads, kb processed in pairs."""
            nkb = 4 * (jt + 1)

            def alloc2_o():
                t = s2ps.tile([128, 2, QTILE], F32, name="s2_ps")
                return [t[:, 0, :], t[:, 1, :]]
            y_tile = ypool.tile([128, H_LOC, QTILE], BF16, name="y_tile")

            state = {}
            staged = []  # (h, kbp, p2, qoffs) awaiting l/y emission

            def flush_one():
                h, kbp, p2, qoffs = staged.pop(0)
                st = state[h]
                kbs = (2 * kbp, 2 * kbp + 1)
                for half, (kb, qoff) in enumerate(zip(kbs, qoffs)):
                    nc.tensor.matmul(
                        st["l_ps"][:, qoff:],
                        ones_sb,
                        p2[:, half, qoff:],
                        start=(kb == 0),
                        stop=(kb == nkb - 1),
                    )
                for half, (kb, qoff) in enumerate(zip(kbs, qoffs)):
                    nc.tensor.matmul(
                        st["y_ps"][:, qoff:],
                        v_sb[:, kb, h * HD : (h + 1) * HD],
                        p2[:, half, qoff:],
                        start=(kb == 0),
                        stop=(kb == nkb - 1),
                    )
                if kbs[1] == nkb - 1:
                    # softmax denominator -> broadcast 1/l -> normalized evac
                    linv = lpool.tile([1, QTILE], F32, name="linv")
                    nc.vector.reciprocal_approx_fast(linv, st["l_ps"])
                    linv_dr = nbounce.tile([1, QTILE], F32, name="linv_dr")
                    nc.sync.dma_start(linv_dr, linv)
                    bc_sb = bcpool.tile([128, QTILE], F32, name="bc_sb")
                    nc.sync.dma_start(bc_sb, linv_dr.to_broadcast([128, QTILE]))
                    nc.vector.tensor_tensor(
                        y_tile[:, h, :], st["y_ps"], bc_sb, ALU.mult
                    )

            steps = 0
            for h in range(H_LOC):
                state[h] = {
                    "y_ps": yps.tile([HD, QTILE], F32, name="y_ps"),
                    "l_ps": lps.tile([1, QTILE], F32, name="l_ps"),
                }
                for kbp in range(nkb // 2):
                    kbs = (2 * kbp, 2 * kbp + 1)
                    s2 = s2ps.tile([128, 2, QTILE], F32, name="s2_ps")
                    qoffs = []
                    for half, kb in enumerate(kbs):
                        s_diag = kb - 4 * jt
                        qoff = 128 * s_diag if s_diag > 0 else 0
                        qoffs.append(qoff)
                        nc.tensor.matmul(
                            s2[:, half, qoff:],
                            kt_sb[:, h, kb * 128 : (kb + 1) * 128],
                            qt_sb[:, h, jt * QTILE + qoff : (jt + 1) * QTILE],
                            start=True,
                            stop=True,
                        )
                    p2 = ppool.tile([128, 2, QTILE], BF16, name="p2")
                    if kbs[1] - 4 * jt < 0:
                        # both halves full: one 1024-wide exp
                        nc.scalar.activation(p2, s2, AF.Exp, scale=SCALE)
                    else:
                        # one wide exp; [0:qoff) of a diag half is stale
                        # psum whose exp is garbage but never read by l/y
                        lo = min(qoffs)
                        if lo == 0:
                            nc.scalar.activation(p2, s2, AF.Exp, scale=SCALE)
                        else:
                            for half, (kb, qoff) in enumerate(zip(kbs, qoffs)):
                                nc.scalar.activation(
                                    p2[:, half, qoff:],
                                    s2[:, half, qoff:],
                                    AF.Exp,
                                    scale=SCALE,
                                )
                        for half, (kb, qoff) in enumerate(zip(kbs, qoffs)):
                            if kb - 4 * jt >= 0:
                                # mask only the 128-wide diagonal triangle
                                nc.gpsimd.affine_select(
                                    out=p2[:, half, qoff : qoff + 128],
                                    in_=p2[:, half, qoff : qoff + 128],
                                    pattern=[[1, 128]],
                                    compare_op=ALU.is_ge,
                                    fill=0.0,
                                    base=0,
                                    channel_multiplier=-1,
                                )
                    staged.append((h, kbp, p2, qoffs))
                    if len(staged) > 3:
                        flush_one()
                    steps += 1
                    if proj_queue and steps % 2 == 0 and steps >= 4:
                        proj_queue.pop(0)(alloc2_o)
            while staged:
                flush_one()
            while proj_queue:
                proj_queue.pop(0)(alloc2_o)
            return y_tile

        def make_proj_thunks(b, jt, y_tile):
            """Projection for tile jt as per-co-pair thunks (8 of them)."""
            tsl = slice(jt * QTILE, (jt + 1) * QTILE)

            def make(cp):
                def emit(alloc2):
                    aps = alloc2()
                    for half in range(2):
                        co = 2 * cp + half
                        for h in range(H_LOC):
                            nc.tensor.matmul(
                                aps[half],
                                wp_sb[:, h, co * 128 : (co + 1) * 128],
                                y_tile[:, h, :],
                                start=(h == 0),
                                stop=(h == H_LOC - 1),
                            )
                    for half in range(2):
                        co = 2 * cp + half
                        o_sb = opool.tile([128, QTILE], F32, name="o_sb")
                        nc.vector.tensor_copy(o_sb, aps[half])
                        nc.sync.dma_start(
                            out_t[b, co * 128 : (co + 1) * 128, tsl], o_sb
                        )

                return emit

            return [make(cp) for cp in range(C // 256)]

        # ---------- main schedule ----------
        def load_x(b, js):
            x_sb = xpool.tile([128, KO, TSLAB], BF16, name="x_sb")
            nc.sync.dma_start(x_sb, x3[b][:, :, js * TSLAB : (js + 1) * TSLAB])
            return x_sb

        x_tiles = {(0, 0): x_first, (0, 1): x_second}
        proj_queue = []
        for b in range(B_LOC):
            with (
                tc.tile_pool(name="qkps", bufs=3, space="PSUM") as qkps,
                tc.tile_pool(name="rotps", bufs=2, space="PSUM") as rotps,
                tc.tile_pool(name="vpsp", bufs=1, space="PSUM") as vpsp,
            ):
                for js in range(NSLAB):
                    g = b * NSLAB + js
                    if g + 1 < B_LOC * NSLAB and (g + 1) != 1:
                        nb, njs = divmod(g + 1, NSLAB)
                        x_tiles[(nb, njs)] = load_x(nb, njs)
                    emit_slab(
                        b, js, x_tiles.pop((b, js)), proj_queue,
                        qkps, rotps, vpsp, first=(g == 0),
                    )
            with (
                tc.tile_pool(name="s2ps", bufs=2, space="PSUM") as s2ps,
                tc.tile_pool(name="yps", bufs=2, space="PSUM") as yps,
                tc.tile_pool(name="lps", bufs=2, space="PSUM") as lps,
            ):
                # last batch: big tile-3 proj zippers into small tile 0,
                # leaving the short tile-0 proj as the kernel tail
                order = range(NQT) if b < B_LOC - 1 else (1, 2, 0, 3)
                for jt in order:
                    y_tile = emit_attn_tile(b, jt, proj_queue, s2ps, yps, lps)
                    proj_queue = make_proj_thunks(b, jt, y_tile)
        # the very last tile's projection: own deep psum scope so the
        # o-accumulate/evac pipeline never waits on a 2-slot ring
        with tc.tile_pool(name="finps", bufs=6, space="PSUM") as finps:
            while proj_queue:
                proj_queue.pop(0)(
                    lambda: [
                        finps.tile([128, QTILE], F32, name="fin_ps"),
                        finps.tile([128, QTILE], F32, name="fin_ps"),
                    ]
                )


def _get_nc():
    if "nc" not in _CACHED:
        _CACHED["nc"] = build_nc()
    return _CACHED["nc"]


def rope_perm_matrix():
    """lhsT for the rotate-half matmul: rot^T = PT.T @ q^T.
    rot[2i] = -q[2i+1], rot[2i+1] = q[2i]."""
    ptm = np.zeros((HD, HD), dtype=np.float32)
    for i in range(HD // 2):
        ptm[2 * i + 1, 2 * i] = -1.0
        ptm[2 * i, 2 * i + 1] = 1.0
    return ptm


def _prep_in_maps(x, sin, cos, W_qkv, W_proj):
    sin_t = np.ascontiguousarray(sin[0, 0].T).astype(np.float32)  # [HD, T]
    cos_t = np.ascontiguousarray(cos[0, 0].T).astype(np.float32)
    ptm = rope_perm_matrix()
    ones_col = np.ones((128, 1), BF16NP)

    in_maps = []
    for g in range(BGROUPS):
        x_tg = np.ascontiguousarray(
            x[g * B_LOC : (g + 1) * B_LOC].transpose(0, 2, 1)
        ).astype(BF16NP)  # [B_LOC, C, T]
        for s in range(HSHARDS):
            qcols = W_qkv[:, s * FQK : (s + 1) * FQK]
            kcols = W_qkv[:, C + s * FQK : C + (s + 1) * FQK]
            vcols = W_qkv[:, 2 * C + s * FV : 2 * C + (s + 1) * FV]
            w_qkv_loc = np.ascontiguousarray(
                np.concatenate([qcols, kcols, vcols], axis=1)
            ).astype(BF16NP)
            w_proj_loc = np.ascontiguousarray(
                W_proj[s * FV : (s + 1) * FV, :]
                .reshape(H_LOC, HD, C)
                .transpose(1, 0, 2)
            ).astype(BF16NP)  # [HD, H_LOC, C]
            in_maps.append(
                {
                    "x_t": x_tg,
                    "w_qkv": w_qkv_loc,
                    "w_proj": w_proj_loc,
                    "sin_t": sin_t,
                    "cos_t": cos_t,
                    "pt": ptm,
                    "ones_col": ones_col,
                }
            )
    return in_maps


def kernel(x, sin, cos, W_qkv, W_proj):
    x = np.asarray(x, dtype=np.float32)
    sin = np.asarray(sin, dtype=np.float32)
    cos = np.asarray(cos, dtype=np.float32)
    W_qkv = np.asarray(W_qkv, dtype=np.float32)
    W_proj = np.asarray(W_proj, dtype=np.float32)

    in_maps = _prep_in_maps(x, sin, cos, W_qkv, W_proj)

    trace = bool(int(os.environ.get("KERNEL_TRACE", "0")))
    if trace:
        _install_ntff_hook()
    nc = _get_nc()
    res = run_bass_kernel_spmd(
        nc, in_maps, core_ids=list(range(NCORES)), trace=trace
    )
    _CACHED["last_result"] = res

    out = np.zeros((B, T, C), dtype=np.float32)
    for g in range(BGROUPS):
        acc = np.zeros((B_LOC, C, T), dtype=np.float32)
        for s in range(HSHARDS):
            acc += res.results[g * HSHARDS + s]["out_t"]
        out[g * B_LOC : (g + 1) * B_LOC] = acc.transpose(0, 2, 1)
    return out
